# revision 26
# baseline (speedup 1.0000x reference)
"""NetVLAD Trainium2 kernel: data-parallel over batch (2 samples/core x 8 cores).

v3: conv1 + conv2 run in fp8 e4m3 with DoubleRow perf mode (2 contraction
rows per PE cycle -> 2x matmul throughput; measured 169 ns per N=400
DR matmul vs 169 ns bf16, double the work).  conv1 pairs taps along ky
(stride-208 pairs) plus two stride-1 kx pairs on the ky=4 row; conv2
pairs the two 128-channel input groups (h1 stored group-interleaved
fp8).  Weights are pre-scaled x16 to dodge e4m3 subnormals; the 1/16
fold happens in the existing bias tensor_scalar.  conv3, soft-assign,
VLAD and the MLP stay bf16/fp32 for accuracy (measured scale-rel err
~1.1e-2 vs 2e-2 budget).
"""

import sys

sys.path.insert(0, "/opt/trn_rl_repo")

from dataclasses import replace as _ap_replace

import numpy as np
import ml_dtypes

BF = ml_dtypes.bfloat16
E4 = ml_dtypes.float8_e4m3

import concourse.bass as bass
import concourse.tile as tile
from concourse import bacc, mybir
from concourse.bass_utils import run_bass_kernel_spmd

F32 = mybir.dt.float32
BF16 = mybir.dt.bfloat16
FP8 = mybir.dt.float8e4
PM = mybir.MatmulPerfMode
ALU = mybir.AluOpType
ACT = mybir.ActivationFunctionType
AXIS = mybir.AxisListType

S = 2          # samples per core
NCORES = 8
C = 128        # input channels
K = 64         # clusters
H, W = 40, 200
P = H * W      # 8000 pixels
HP, WP = H + 4, W + 4          # padded 44 x 204 (bf16 soft-assign copy)
W8 = 208                       # fp8 conv1 copy padded to 208-wide rows
CHUNK = 100                     # pixels per vlad chunk (half of a row)
NCHUNK = P // CHUNK             # 80
NB = NCHUNK // 8                # 10 batches of 8 chunks
EPS = 1e-12
LEAK = 0.2
WSC = 16.0                     # fp8 weight pre-scale
WSCI = 1.0 / WSC

# conv1: 128 -> 256, conv2: 256 -> 512, conv3: 512 -> 1024
# h1 pooled 20x100 (padded 24x104), h2 pooled 5x25 (padded 9x32: 32-wide
# rows keep the fp8 group-pair stride 16B-aligned and let the conv3
# moving operand over-read to 26 columns)
H1, W1 = 20, 100
H1P, W1P = 24, 104
H2, W2 = 5, 25
H2P, W2P = 9, 32
FEAT_CHUNKS = 73  # 64 vlad + 8 xfeat + 1 bias row
N_EARLY = 8       # conv1 (s=0, c2=0) groups pre-SA-hook filled into phase A
# (sample, b) -> t20 for tail-position fillers (emitted before the vlad block)
# phase-A fillers: DoubleRow groups early in sample 1's SA pipeline
# hard-fault the device (bisected empirically: DR fillers pass only with
# <=1 SA batch still ahead).  So early fillers run in normal fp8 mode
# (same weights read unpaired, 25 MMs) and only the late positions use DR.
import os as _os
_XV = _os.environ.get("KXV", "x1")
if _XV == "x1":      # safe tail fills only, skip exactly those groups in B
    TAIL_FILL = {(0, 10): (4, "dr"), (0, 11): (5, "dr"),
                 (1, 9): (6, "dr"), (1, 11): (7, "dr")}
    SKIP_SET = {4, 5, 6, 7}
elif _XV == "x3":    # two groups per safe tail position
    TAIL_FILL = {(0, 10): [(4, "dr"), (0, "dr")], (0, 11): [(5, "dr"), (1, "dr")],
                 (1, 9): [(6, "dr"), (2, "dr")], (1, 11): [(7, "dr"), (3, "dr")]}
    SKIP_SET = {0, 1, 2, 3, 4, 5, 6, 7}
elif _XV == "x2":    # tail fills, no skip (duplicate emission) — known good
    TAIL_FILL = {(0, 10): (4, "dr"), (0, 11): (5, "dr"),
                 (1, 9): (6, "dr"), (1, 11): (7, "dr")}
    SKIP_SET = set()
else:                # no fills at all
    TAIL_FILL = {}
    SKIP_SET = set()

# conv1 tap pairing: 10 ky-pairs (stride W8) + 2 kx-pairs on ky=4 (stride 1)
# + 1 single (ky=4, kx=4).  pair p<10: kx=p//2, ky0=2*(p%2); p=10: (4,0)+(4,1);
# p=11: (4,2)+(4,3).


def _strided(apv, dims):
    """AP with explicit [stride, n] free dims after the partition dim."""
    return _ap_replace(
        apv, ap=type(apv.ap)([list(apv.ap[0])] + [list(d) for d in dims]))


def build_program():
    nc = bacc.Bacc("TRN2", target_bir_lowering=False)

    # ---- DRAM I/O ----
    d_xsa = nc.dram_tensor("xsa", [S, C, P], BF16, kind="ExternalInput")
    d_xp8 = nc.dram_tensor("xp8", [S, C, HP * W8], FP8, kind="ExternalInput")
    d_xt = nc.dram_tensor("xt", [S, CHUNK, NCHUNK * 130], BF16, kind="ExternalInput")
    d_w1d = nc.dram_tensor("w1d", [C, 2 * 12 * 2 * 128], FP8, kind="ExternalInput")
    d_w1s = nc.dram_tensor("w1s", [C, 2 * 128], FP8, kind="ExternalInput")
    d_w2d = nc.dram_tensor("w2d", [4, C, 25 * 2 * 128], FP8, kind="ExternalInput")
    d_w3d = nc.dram_tensor("w3d", [8, 2, C, 25 * 2 * 128], FP8, kind="ExternalInput")
    d_wsa = nc.dram_tensor("wsa", [C, K], BF16, kind="ExternalInput")
    d_bsab = nc.dram_tensor("bsab", [128, K], F32, kind="ExternalInput")
    d_b1 = nc.dram_tensor("b1r", [128, 2], F32, kind="ExternalInput")
    d_b2 = nc.dram_tensor("b2r", [128, 4], F32, kind="ExternalInput")
    d_b3 = nc.dram_tensor("b3r", [128, 8], F32, kind="ExternalInput")
    d_cent = nc.dram_tensor("cent", [K, C], F32, kind="ExternalInput")
    d_mlpw = nc.dram_tensor("mlpwt", [19, 128, 4 * 256], BF16, kind="ExternalInput")
    d_out = nc.dram_tensor("out", [S, 256], F32, kind="ExternalOutput")
    d_ident = nc.inline_tensor(np.eye(128, dtype=np.float32).astype(BF),
                               name="ident")
    _brow = np.zeros((128, 2), np.float32)
    _brow[0, :] = 1.0
    d_brow = nc.inline_tensor(_brow.astype(BF), name="brow")

    with tile.TileContext(nc) as tc:
        _body(nc, tc, d_xsa, d_xp8, d_xt, d_w1d, d_w1s, d_w2d, d_w3d,
              d_wsa, d_bsab, d_b1, d_b2, d_b3, d_cent, d_mlpw, d_out,
              d_ident, d_brow)

    nc.compile()
    return nc


def _body(nc, tc, d_xsa, d_xp8, d_xt, d_w1d, d_w1s, d_w2d, d_w3d,
          d_wsa, d_bsab, d_b1, d_b2, d_b3, d_cent, d_mlpw, d_out,
          d_ident, d_brow):
    from contextlib import ExitStack

    # ---- pool creation in stack (LIFO) order; DMAs issued afterwards in
    # priority order (x streams first on sync, consts on scalar) ----
    es_const = ExitStack()
    const = es_const.enter_context(tc.tile_pool(name="const", bufs=1))
    wsa_sb = const.tile([C, K], BF16, tag="wsa")
    bsab_sb = const.tile([128, K], F32, tag="bsab")
    cent_sb = const.tile([K, C], F32, tag="cent")
    b1_sb = const.tile([128, 2], F32, tag="b1")
    b2_sb = const.tile([128, 4], F32, tag="b2")
    b3_sb = const.tile([128, 8], F32, tag="b3")
    ident_sb = const.tile([128, 128], BF16, tag="ident")
    ones64_sb = const.tile([K, K], BF16, tag="ones64")
    ones128_sb = const.tile([128, 128], BF16, tag="ones128")
    biasrow_sb = const.tile([128, 2], BF16, tag="biasrow")
    # feature tiles for the MLP (filled by phases A and D)
    fv_sb = const.tile([128, K * S], BF16, tag="fv")      # [c, (k, s)]
    xf_sb = const.tile([128, S * 8], F32, tag="xf")      # [c, (s, c8)]

    es_ab = ExitStack()
    xpad_pool = es_ab.enter_context(tc.tile_pool(name="xpad", bufs=S))
    xp8_pool = es_ab.enter_context(tc.tile_pool(name="xp8", bufs=S))
    w1_pool = es_ab.enter_context(tc.tile_pool(name="w1p", bufs=2))
    # pools that survive into phase B (vlad posts + interleaved conv1
    # groups run alongside phase A / under conv1)
    es_apost = ExitStack()
    vpost_pool = es_apost.enter_context(tc.tile_pool(name="vpost", bufs=2))
    vlad_ps = es_apost.enter_context(
        tc.tile_pool(name="vlad_ps", bufs=2, space="PSUM"))
    tp_ps = es_apost.enter_context(tc.tile_pool(name="tp_ps", bufs=1, space="PSUM"))
    # conv1 pools created before phase A so conv1 groups can interleave
    # into phase-A stall windows (PSUM: vlad 2 + tp 2 + c1 2 + sa 2 = 8)
    c1_ps = es_apost.enter_context(tc.tile_pool(name="c1ps", bufs=2, space="PSUM"))
    pb_pool = es_apost.enter_context(tc.tile_pool(name="poolb", bufs=4))
    xt_pool_l = ExitStack()
    xt_pool = xt_pool_l.enter_context(tc.tile_pool(name="xtl", bufs=S))

    es_bd = ExitStack()  # mlp weights live from B to E (bottom of right stack)
    mw_pool = es_bd.enter_context(
        tc.tile_pool(name="mws", bufs=19, side="right"))
    es_bc = ExitStack()  # right-side pools live through B and C
    h1_pool = es_bc.enter_context(
        tc.tile_pool(name="h1", bufs=S, side="right"))
    w2_pool = es_bc.enter_context(
        tc.tile_pool(name="w2s", bufs=4, side="right"))

    # h1: per sample, both 128-channel groups interleaved for conv2
    # DoubleRow pairs: [c, (c2, pix)] fp8, pair stride H1P*W1P = 2496 B.
    # The memsets sit BEFORE the xt chunk DMAs on the gpsimd queue on
    # purpose: delaying xt ~9us serializes it behind the xpad stream,
    # which otherwise contends for HBM exactly while soft-assign is
    # racing its xpad chunk arrivals (measured +4us of PE stall).
    h1_sb = [None] * S
    for s in range(S):
        t = h1_pool.tile([128, 2 * H1P * W1P], FP8, tag="h1", name=f"h1_{s}")
        nc.gpsimd.memset(t[:], 0.0)
        h1_sb[s] = t

    # ---- input streams in consumption order: first x-pad row-chunks of
    # sample 0 + the soft-assign weights, then the rest ----
    xsa_sb = [xpad_pool.tile([C, P], BF16, tag="xsa", name=f"xp{s}")
              for s in range(S)]
    xp8_sb = [xp8_pool.tile([C, HP * W8], FP8, tag="xp8", name=f"x8{s}")
              for s in range(S)]
    xt_sb = [xt_pool.tile([CHUNK, NCHUNK * 130], BF16, tag="xt", name=f"xt{s}")
             for s in range(S)]

    # raw (unpadded) rows: soft-assign never reads the conv padding, so
    # the SA copy ships unpadded (0.3 MB/sample less on the critical
    # startup stream; SA batch b needs rows <= 4b+3)
    row_chunks = [(0, 4), (4, 15), (15, 26), (26, 36), (36, 40)]

    def xpad_chunk(s, r):
        a, b = row_chunks[r][0] * W, row_chunks[r][1] * W
        nc.sync.dma_start(xsa_sb[s][:, a:b], d_xsa[s][:, a:b])

    def xt_chunk(s, g):
        a, b = g * 20 * 130, (g + 1) * 20 * 130
        nc.gpsimd.dma_start(xt_sb[s][:, a:b], d_xt[s][:, a:b])

    # interleaved in consumption order: SA batch b needs xpad rows <= 4b+5,
    # vlad batch b needs xt group b//2.5 -- small first chunk so the first
    # soft-assign matmul issues as early as possible
    w1d_sb = w1_pool.tile([C, 2 * 12 * 2 * 128], FP8, tag="w1d")
    w1s_sb = w1_pool.tile([C, 2 * 128], FP8, tag="w1s")

    nc.sync.dma_start(wsa_sb[:], d_wsa[:])
    nc.scalar.dma_start(bsab_sb[:], d_bsab[:])
    for s in range(S):
        xpad_chunk(s, 0)
        xpad_chunk(s, 1)
        xt_chunk(s, 0)
        xpad_chunk(s, 2)
        xt_chunk(s, 1)
        xpad_chunk(s, 3)
        xt_chunk(s, 2)
        xpad_chunk(s, 4)
        xt_chunk(s, 3)
        if s == 0:
            # w1 + conv1 bias + fp8 x land before sample 1's phase A so
            # conv1 groups can interleave into its stall windows; scalar
            # queue so the xpad/xt streams keep their own bandwidth
            # (NOTE: do NOT move these onto sync behind the xpad chunks --
            # the delayed arrival re-triggers the phase-A fill device
            # fault, NRT_EXEC_UNIT_UNRECOVERABLE)
            nc.scalar.dma_start(xp8_sb[0][:], d_xp8[0][:])
            nc.scalar.dma_start(w1d_sb[:], d_w1d[:])
            nc.scalar.dma_start(w1s_sb[:], d_w1s[:])
            nc.scalar.dma_start(b1_sb[:], d_b1[:])
    nc.scalar.dma_start(cent_sb[:], d_cent[:])
    nc.scalar.dma_start(b2_sb[:], d_b2[:])
    nc.scalar.dma_start(b3_sb[:], d_b3[:])
    nc.scalar.dma_start(ident_sb[:], d_ident[:])
    nc.gpsimd.memset(ones64_sb[:], 1.0)
    nc.gpsimd.memset(ones128_sb[:], 1.0)
    nc.scalar.dma_start(biasrow_sb[:], d_brow[:])

    w1dv = w1d_sb[:].rearrange("p (g t two o) -> p g t two o", g=2, t=12, two=2)
    w1sv = w1s_sb[:].rearrange("p (g o) -> p g o", g=2)

    # one conv1 output group (2 output rows): 12 DoubleRow fp8 pair-matmuls
    # + 1 single-tap fp8 matmul, 2x2 maxpool, bias (x16 weights folded back
    # with the 1/16), leaky, store into group-interleaved fp8 h1.
    def conv1_group(s, c2, t20, mode="dr"):
        xv8 = xp8_sb[s][:].rearrange("p (h w) -> p h w", h=HP)
        r0 = 2 * t20
        ps = c1_ps.tile([128, 400], F32, tag="c1")
        if mode == "dr":
            for p in range(12):
                if p < 10:
                    kx, ky0 = p // 2, 2 * (p % 2)
                    base = xv8[:, r0 + ky0:r0 + ky0 + 1, kx:kx + 1]
                    rhs = _strided(base, [[W8, 2], [W8, 2], [1, 200]])
                else:
                    kx0 = (p - 10) * 2
                    base = xv8[:, r0 + 4:r0 + 5, kx0:kx0 + 1]
                    rhs = _strided(base, [[1, 2], [W8, 2], [1, 200]])
                nc.tensor.matmul(ps[:], w1dv[:, c2, p], rhs,
                                 start=(p == 0), stop=False,
                                 perf_mode=PM.DoubleRow)
        else:
            for p in range(12):
                for i in range(2):
                    if p < 10:
                        ky, kx = 2 * (p % 2) + i, p // 2
                    else:
                        ky, kx = 4, (p - 10) * 2 + i
                    nc.tensor.matmul(
                        ps[:], w1dv[:, c2, p, i],
                        xv8[:, r0 + ky:r0 + ky + 2, kx:kx + 200],
                        start=(p == 0 and i == 0), stop=False)
        nc.tensor.matmul(ps[:], w1sv[:, c2], xv8[:, r0 + 4:r0 + 6, 4:204],
                         start=False, stop=True)
        # maxpool 2x2 -> [128, 100], then bias + leaky
        m1 = pb_pool.tile([128, 200], F32, tag="m1")
        nc.vector.tensor_reduce(
            m1[:], ps[:].rearrange("p (a two) -> p a two", two=2),
            axis=AXIS.X, op=ALU.max)
        m2 = pb_pool.tile([128, 100], F32, tag="m2")
        mv = m1[:].rearrange("p (r x) -> p r x", r=2)
        nc.vector.tensor_tensor(
            m2[:].rearrange("p (r x) -> p r x", r=1),
            mv[:, 0:1, :], mv[:, 1:2, :], op=ALU.max)
        nc.vector.tensor_scalar(
            m2[:], m2[:], b1_sb[:, c2:c2 + 1], WSCI, op0=ALU.add, op1=ALU.mult)
        dst = h1_sb[s][:, c2 * (H1P * W1P) + (t20 + 2) * W1P + 2:
                       c2 * (H1P * W1P) + (t20 + 2) * W1P + 2 + 100]
        nc.vector.scalar_tensor_tensor(
            dst, m2[:], LEAK, m2[:], op0=ALU.mult, op1=ALU.max)

    # ---------------- Phase A: soft-assign + VLAD ----------------
    es_a = ExitStack()
    ea_pool = es_a.enter_context(tc.tile_pool(name="ea", bufs=4))
    u_pool = es_a.enter_context(tc.tile_pool(name="u", bufs=4))
    sa_ps = es_a.enter_context(tc.tile_pool(name="sa_ps", bufs=2, space="PSUM"))

    vps_l = [None] * S
    for s in range(S):
        xpv = xsa_sb[s]
        xtv = xt_sb[s]
        vps = vlad_ps.tile([K, 132], F32, tag="vlad")
        vps_l[s] = vps
        # 2-deep software pipeline: soft-assign batch b, vlad batch b-2, so
        # the ~2us softmax chain latency hides under two batches of PE work
        a_hist = {}
        for b in range(NB + 2):
            # tail filler BEFORE the vlad block: fills xt/softmax stalls in
            # the thin tail iterations (only 8 vlad matmuls of PE work each)
            if (s, b) in TAIL_FILL:
                fills = TAIL_FILL[(s, b)]
                if not isinstance(fills, list):
                    fills = [fills]
                for ft, fm in fills:
                    conv1_group(0, 0, ft, mode=fm)
            if b < NB:
                saps = sa_ps.tile([128, 512], F32, tag="sa")
                for i8 in range(8):
                    ci = b * 8 + i8
                    y, half = divmod(ci, 2)
                    off = y * W + half * CHUNK
                    nc.tensor.matmul(
                        saps[0:CHUNK, i8 * K:(i8 + 1) * K],
                        xpv[:, off:off + CHUNK],
                        wsa_sb[:],
                        start=True, stop=True,
                    )
                e_t = ea_pool.tile([128, 512], F32, tag="e")
                nc.vector.tensor_tensor(
                    e_t[0:CHUNK].rearrange("p (a b) -> p a b", a=8),
                    saps[0:CHUNK].rearrange("p (a b) -> p a b", a=8),
                    bsab_sb[0:CHUNK].rearrange("p (o k) -> p o k", o=1).broadcast_to((CHUNK, 8, K)),
                    op=ALU.add,
                )
                nc.scalar.activation(e_t[0:CHUNK], e_t[0:CHUNK], ACT.Exp)
                ssum = u_pool.tile([128, 8], F32, tag="ssum")
                nc.vector.tensor_reduce(
                    ssum[0:CHUNK], e_t[0:CHUNK].rearrange("p (a b) -> p a b", a=8),
                    axis=AXIS.X, op=ALU.add,
                )
                u_t = u_pool.tile([128, 8], F32, tag="u")
                nc.vector.reciprocal(u_t[0:CHUNK], ssum[0:CHUNK])
                a_t = ea_pool.tile([128, 512], BF16, tag="a")
                nc.vector.tensor_tensor(
                    a_t[0:CHUNK].rearrange("p (a b) -> p a b", a=8),
                    e_t[0:CHUNK].rearrange("p (a b) -> p a b", a=8),
                    u_t[0:CHUNK].rearrange("p (a o) -> p a o", o=1).broadcast_to((CHUNK, 8, K)),
                    op=ALU.mult,
                )
                a_hist[b] = a_t
            if b >= 2:
                bb = b - 2
                at = a_hist.pop(bb)
                for i8 in range(8):
                    ci = bb * 8 + i8
                    nc.tensor.matmul(
                        vps[0:K, 0:130],
                        at[0:CHUNK, i8 * K:(i8 + 1) * K],
                        xtv[:, ci * 130:(ci + 1) * 130],
                        start=(ci == 0), stop=(ci == NCHUNK - 1),
                    )
    # vlad post for both samples, deferred into phase B: the PE ops (gps,
    # transpose) queue behind a few conv1 groups so their DVE input chains
    # complete long before the PE reaches them
    def do_vlad_post(s):
        vps = vps_l[s]
        # vlad post: v' = centers*A - vlad1  (negated vlad)
        vp = vpost_pool.tile([K, C], F32, tag="vp")
        acol = vpost_pool.tile([K, 4], F32, tag="acol")
        nc.vector.tensor_copy(acol[:, 0:1], vps[0:K, 128:129])
        nc.vector.scalar_tensor_tensor(
            vp[:], cent_sb[:], acol[:, 0:1], vps[0:K, 0:C],
            op0=ALU.mult, op1=ALU.subtract,
        )
        sq = vpost_pool.tile([K, C], F32, tag="vsq")
        ssk = vpost_pool.tile([K, 4], F32, tag="ssk")
        nc.scalar.activation(sq[:], vp[:], ACT.Square, accum_out=ssk[:, 0:1])
        nc.scalar.sqrt(ssk[:, 1:2], ssk[:, 0:1])
        nc.vector.tensor_scalar_max(ssk[:, 1:2], ssk[:, 1:2], EPS)
        nc.vector.reciprocal(ssk[:, 2:3], ssk[:, 1:2])
        # per-row ss of the normalized rows = ssk * ik^2
        nc.vector.tensor_scalar(
            ssk[:, 3:4], ssk[:, 0:1], ssk[:, 2:3], None, op0=ALU.mult)
        nc.vector.tensor_scalar(
            ssk[:, 3:4], ssk[:, 3:4], ssk[:, 2:3], None, op0=ALU.mult)
        sskb = vpost_pool.tile([K, 2], BF16, tag="sskb")
        nc.vector.tensor_copy(sskb[:, 0:1], ssk[:, 3:4])
        gps = tp_ps.tile([K, 4], F32, tag="gps")
        nc.tensor.matmul(gps[:, 0:1], ones64_sb[:], sskb[:, 0:1],
                         start=True, stop=True)
        gsb = vpost_pool.tile([K, 4], F32, tag="gsb")
        nc.scalar.sqrt(gsb[:, 0:1], gps[:, 0:1])
        nc.vector.tensor_scalar_max(gsb[:, 0:1], gsb[:, 0:1], EPS)
        nc.vector.reciprocal(gsb[:, 1:2], gsb[:, 0:1])
        # combined scale = -ik * ginv
        nc.vector.tensor_scalar(
            gsb[:, 2:3], ssk[:, 2:3], gsb[:, 1:2], -1.0,
            op0=ALU.mult, op1=ALU.mult)
        vf = vpost_pool.tile([K, C], BF16, tag="vf")
        nc.vector.tensor_scalar(vf[:], vp[:], gsb[:, 2:3], None, op0=ALU.mult)
        # transpose [64, 128] -> [128, 64] and store into fv[:, (k, s)]
        vtps = tp_ps.tile([128, K], BF16, tag="vt")
        nc.tensor.transpose(vtps[:], vf[:], ident_sb[0:K, 0:K])
        nc.vector.tensor_copy(
            fv_sb[:].rearrange("p (k s) -> p k s", s=S)[:, :, s],
            vtps[:],
        )
    # xp8[1] is first consumed mid-phase-B: its doorbell sits after the
    # phase-A loop so the scalar engine (busy with softmax Exp) only
    # fires it ~55us in, keeping startup HBM bandwidth for the SA stream
    nc.scalar.dma_start(xp8_sb[1][:], d_xp8[1][:])
    es_a.close()
    xt_pool_l.close()

    # ---------------- Phase B: conv1 + pool 2x2 + leaky ----------------
    w2_sb = {}

    def load_w2(c4):
        t = w2_pool.tile([128, 25 * 2 * 128], FP8, tag="w2s")
        nc.sync.dma_start(t[:], d_w2d[c4])
        w2_sb[c4] = t

    load_w2(0)

    mw_sb = {}

    def load_mw(g):
        t = mw_pool.tile([128, 4 * 256], BF16, tag="mws")
        nc.sync.dma_start(t[:], d_mlpw[g])
        mw_sb[g] = t

    for s in range(S):
        if s == 1:
            for g in range(19):
                load_mw(g)
        for c2 in range(2):
            for t20 in range(20):
                if s == 0 and c2 == 0 and t20 in SKIP_SET:
                    continue  # already emitted interleaved into phase A
                if s == 0 and c2 == 0 and t20 == N_EARLY + 3:
                    do_vlad_post(0)
                    do_vlad_post(1)
                conv1_group(s, c2, t20)
    es_apost.close()
    es_ab.close()

    # ---------------- Phase C: conv2 + pool 4x4 + leaky ----------------
    # one PSUM pool shared by conv2, conv3 and the MLP so bank rotation
    # pipelines across the phase boundaries (no bank-drain wait at C->D->E)
    es_cde = ExitStack()
    cd_ps = es_cde.enter_context(tc.tile_pool(name="cdps", bufs=6, space="PSUM"))
    mlp_ps = es_cde.enter_context(tc.tile_pool(name="mlpps", bufs=1, space="PSUM"))
    nm_ps = es_cde.enter_context(tc.tile_pool(name="nmps", bufs=1, space="PSUM"))
    es_cd = ExitStack()  # left-side pools live through C and D
    h2_pool = es_cd.enter_context(tc.tile_pool(name="h2", bufs=4))
    w3_pool = es_cd.enter_context(tc.tile_pool(name="w3s", bufs=16))
    es_c = ExitStack()
    pc_pool = es_c.enter_context(tc.tile_pool(name="poolc", bufs=4))

    # h2: tile j holds conv2 output groups (2j, 2j+1) as DoubleRow pairs
    # for conv3; pair stride S*H2P*W2P = 576 B
    h2_sb = [None] * 2
    for j in range(2):
        t = h2_pool.tile([128, 2 * S * H2P * W2P], FP8, tag="h2")
        nc.gpsimd.memset(t[:], 0.0)
        h2_sb[j] = t

    w3_sb = {}

    def load_w3(c8):
        for j in range(2):
            t = w3_pool.tile([128, 25 * 2 * 128], FP8, tag="w3s")
            nc.sync.dma_start(t[:], d_w3d[c8, j])
            w3_sb[(c8, j)] = t

    for c4 in range(4):
        if c4 + 1 < 4:
            load_w2(c4 + 1)
        if c4 == 2:
            load_w3(0)
        elif c4 == 3:
            load_w3(1)
            load_w3(2)
        w2v = w2_sb[c4][:].rearrange("p (t two o) -> p t two o", t=25, two=2)
        for s in range(S):
            for rg in range(5):
                ps = cd_ps.tile([128, 400], F32, tag="c23")
                for tap in range(25):
                    ky, kx = divmod(tap, 5)
                    base = h1_sb[s][:, (4 * rg + ky) * W1P + kx:
                                    (4 * rg + ky) * W1P + kx + 1]
                    rhs = _strided(
                        base, [[H1P * W1P, 2], [W1P, 4], [1, 100]])
                    nc.tensor.matmul(
                        ps[:], w2v[:, tap], rhs,
                        start=(tap == 0), stop=(tap == 24),
                        perf_mode=PM.DoubleRow,
                    )
                # maxpool 4x4 over [4 rows, 100] -> [128, 25]
                ma = pc_pool.tile([128, 200], F32, tag="ma")
                nc.vector.tensor_reduce(
                    ma[:], ps[:].rearrange("p (a two) -> p a two", two=2),
                    axis=AXIS.X, op=ALU.max)
                mb = pc_pool.tile([128, 100], F32, tag="mb")
                mav = ma[:].rearrange("p (a two) -> p a two", two=2)
                nc.vector.tensor_tensor(
                    mb[:].rearrange("p (a o) -> p a o", o=1),
                    mav[:, :, 0:1], mav[:, :, 1:2], op=ALU.max)
                mc = pc_pool.tile([128, 50], F32, tag="mc")
                mbv = mb[:].rearrange("p (r x) -> p r x", r=4)
                mcv = mc[:].rearrange("p (r x) -> p r x", r=2)
                nc.vector.tensor_tensor(
                    mcv[:, 0:1, :], mbv[:, 0:1, :], mbv[:, 1:2, :], op=ALU.max)
                nc.vector.tensor_tensor(
                    mcv[:, 1:2, :], mbv[:, 2:3, :], mbv[:, 3:4, :], op=ALU.max)
                md = pc_pool.tile([128, 25], F32, tag="md")
                nc.vector.tensor_tensor(
                    md[:].rearrange("p (r x) -> p r x", r=1),
                    mcv[:, 0:1, :], mcv[:, 1:2, :], op=ALU.max)
                nc.vector.tensor_scalar(
                    md[:], md[:], b2_sb[:, c4:c4 + 1], WSCI,
                    op0=ALU.add, op1=ALU.mult)
                off = (c4 % 2) * (S * H2P * W2P) \
                    + (rg + 2) * (S * W2P) + s * W2P + 2
                nc.vector.scalar_tensor_tensor(
                    h2_sb[c4 // 2][:, off:off + 25], md[:], LEAK, md[:],
                    op0=ALU.mult, op1=ALU.max)
    es_c.close()
    es_bc.close()

    # ---------------- Phase D: conv3 + pool 5x25 ----------------
    es_d = ExitStack()
    pd_pool = es_d.enter_context(tc.tile_pool(name="poold", bufs=4))

    # the MLP's 64 vlad chunks (inputs ready since phase B) interleave 8
    # per c8 group: conv3 is LDWEIGHTS-bound (135ns load vs 113ns of
    # streaming per pair-tap), and the PE's reorder window keeps the LDW
    # chain back-to-back while these fill the ~22ns/pair streaming bubbles
    ops = mlp_ps.tile([S, 256], F32, tag="mlpo")
    fvv = fv_sb[:].rearrange("p (k s) -> p k s", s=S)

    for c8 in range(8):
        if c8 + 3 < 8:
            load_w3(c8 + 3)
        # h2 rows are sample-interleaved (row stride S*W2P, sample W2P),
        # so (row, sample) merges into ONE 10-step stride-32 AP dim and a
        # single N=260 matmul serves both samples per pair-tap: the 135ns
        # DR weight load is the only bound (two N=130 MMs measured
        # 82.5 ns/MM vs the 67.5 floor -- ~12us of second-MM overhead)
        ps_t = cd_ps.tile([128, 400], F32, tag="c23")
        ps = ps_t[:, 0:260]
        for j in range(2):
            wv = w3_sb[(c8, j)][:].rearrange("p (t two o) -> p t two o",
                                             t=25, two=2)
            for tap in range(25):
                ky, kx = divmod(tap, 5)
                base = h2_sb[j][:, ky * (S * W2P) + kx:
                                ky * (S * W2P) + kx + 1]
                rhs = _strided(
                    base, [[S * H2P * W2P, 2], [W2P, S * 5], [1, 26]])
                nc.tensor.matmul(
                    ps, wv[:, tap], rhs,
                    start=(j == 0 and tap == 0),
                    stop=(j == 1 and tap == 24),
                    perf_mode=PM.DoubleRow,
                )
        pv = ps.rearrange("p (h s w) -> p h s w", h=5, s=S)
        for s in range(S):
            mx = pd_pool.tile([128, 4], F32, tag="mx")
            nc.vector.tensor_reduce(
                mx[:, 0:1], pv[:, :, s, 0:25], axis=AXIS.XY, op=ALU.max)
            nc.vector.tensor_scalar(
                xf_sb[:].rearrange("p (s c) -> p s c", s=S)[:, s, c8:c8 + 1],
                mx[:, 0:1], b3_sb[:, c8:c8 + 1], WSCI,
                op0=ALU.add, op1=ALU.mult)
        for i8m in range(8):
            j = c8 * 8 + i8m
            g, i4 = divmod(j, 4)
            nc.tensor.matmul(
                ops[:], fvv[:, j, :], mw_sb[g][:, i4 * 256:(i4 + 1) * 256],
                start=(j == 0), stop=False)
    es_d.close()
    es_cd.close()

    # ---------------- Phase E: x_feat norm + MLP + final norm ----------------
    es_e = ExitStack()
    pe_pool = es_e.enter_context(tc.tile_pool(name="poole", bufs=1))

    # vlad-part MLP chunks first: they only need fv, so the PE starts on
    # them right after conv3; the x_feat norm chain (DVE/ACT + one small
    # matmul) is emitted mid-loop so it overlaps the remaining chunks
    # x_feat l2 norm across the 1024 conv3 features of each sample
    sq = pe_pool.tile([128, S * 8], BF16, tag="sq")
    nc.vector.tensor_tensor(sq[:], xf_sb[:], xf_sb[:], op=ALU.mult)
    sps = nm_ps.tile([128, S * 8], F32, tag="sps")
    nc.tensor.matmul(sps[:], ones128_sb[:], sq[:], start=True, stop=True)

    ssn = pe_pool.tile([128, S * 4], F32, tag="ssn")
    nc.vector.tensor_reduce(
        ssn[:, 0:S], sps[:].rearrange("p (s c) -> p s c", s=S),
        axis=AXIS.X, op=ALU.add)
    nc.scalar.sqrt(ssn[:, S:2 * S], ssn[:, 0:S])
    nc.vector.tensor_scalar_max(ssn[:, S:2 * S], ssn[:, S:2 * S], EPS)
    nc.vector.reciprocal(ssn[:, 2 * S:3 * S], ssn[:, S:2 * S])
    xff = pe_pool.tile([128, S * 8], BF16, tag="xff")
    nc.vector.tensor_tensor(
        xff[:].rearrange("p (s c) -> p s c", s=S),
        xf_sb[:].rearrange("p (s c) -> p s c", s=S),
        ssn[:, 2 * S:3 * S].rearrange("p (s o) -> p s o", s=S).broadcast_to((128, S, 8)),
        op=ALU.mult)

    xfv = xff[:].rearrange("p (s c) -> p s c", s=S)
    for j in range(K, FEAT_CHUNKS):
        lhs = xfv[:, :, j - K] if j < K + 8 else biasrow_sb[:]
        g, i4 = divmod(j, 4)
        nc.tensor.matmul(
            ops[:],
            lhs,
            mw_sb[g][:, i4 * 256:(i4 + 1) * 256],
            start=False, stop=(j == FEAT_CHUNKS - 1),
        )
    sqo = pe_pool.tile([S, 256], F32, tag="sqo")
    nrm = pe_pool.tile([S, 4], F32, tag="nrm")
    nc.scalar.activation(sqo[:], ops[:], ACT.Square, accum_out=nrm[:, 0:1])
    nc.scalar.sqrt(nrm[:, 1:2], nrm[:, 0:1])
    nc.vector.tensor_scalar_max(nrm[:, 1:2], nrm[:, 1:2], EPS)
    nc.vector.reciprocal(nrm[:, 2:3], nrm[:, 1:2])
    out_sb = pe_pool.tile([S, 256], F32, tag="outsb")
    nc.vector.tensor_scalar(
        out_sb[:], ops[:], nrm[:, 2:3], None, op0=ALU.mult)
    nc.sync.dma_start(d_out[:], out_sb[:])
    es_e.close()
    es_cde.close()
    es_bd.close()
    es_const.close()


_PROGRAM = None


def _get_program():
    global _PROGRAM
    if _PROGRAM is None:
        _PROGRAM = build_program()
    return _PROGRAM


def prep_inputs(x, cluster_centers, conv_w, conv_b, w1, b1, w2, b2, w3, b3,
                mlp_w, mlp_b):
    """Host-side re-layout. Returns per-core input dict list."""
    N = x.shape[0]
    x = np.asarray(x, np.float32)
    xsa = x.reshape(N, C, P).astype(BF)
    # fp8 conv1 copy, rows padded to 208 (DoubleRow pair stride alignment)
    xp8 = np.pad(x, ((0, 0), (0, 0), (2, 2), (2, 6))).reshape(N, C, HP * W8)
    xp8 = xp8.astype(E4)
    # xt: [N, 100, 80*130] pixel-transposed x with ones column
    xt = np.ascontiguousarray(x.transpose(0, 2, 3, 1))           # [N, 40, 200, 128]
    xt = xt.reshape(N, NCHUNK, CHUNK, C)                         # chunk = (y, half)
    pad_cols = np.zeros((N, NCHUNK, CHUNK, 2), np.float32)
    pad_cols[..., 0] = 1.0
    xt = np.concatenate([xt, pad_cols], axis=3)                  # [N, 80, 100, 130]
    xt = np.ascontiguousarray(
        xt.transpose(0, 2, 1, 3).reshape(N, CHUNK, NCHUNK * 130)).astype(BF)

    # conv1 DoubleRow weight pairs: w1d[c, c2, p, i, o]; taps per pair p:
    # p<10: (2*(p%2)+i, p//2); p=10: (4, i); p=11: (4, 2+i). single: (4,4).
    w1f = np.asarray(w1, np.float32).reshape(2, 128, C, 5, 5) * WSC
    w1d = np.zeros((C, 2, 12, 2, 128), np.float32)
    for p in range(12):
        for i in range(2):
            if p < 10:
                ky, kx = 2 * (p % 2) + i, p // 2
            else:
                ky, kx = 4, (p - 10) * 2 + i
            w1d[:, :, p, i, :] = w1f[:, :, :, ky, kx].transpose(2, 0, 1)
    w1s = np.ascontiguousarray(
        w1f[:, :, :, 4, 4].transpose(2, 0, 1)).astype(E4)        # [C, 2, 128]
    w1d = np.ascontiguousarray(w1d.reshape(C, -1)).astype(E4)

    # conv2 DoubleRow: w2d[c4, c_in_grp, tap, grp, o]
    w2f = np.asarray(w2, np.float32).reshape(4, 128, 2, 128, 5, 5) * WSC
    w2d = np.ascontiguousarray(
        w2f.transpose(3, 4, 5, 2, 0, 1)                          # [c,ky,kx,grp,c4,o]
        .reshape(128, 25, 2, 4, 128).transpose(3, 0, 1, 2, 4)
        .reshape(4, 128, 25 * 2 * 128)).astype(E4)
    # conv3 DoubleRow pairs: w3d[c8, j, c, tap, i, o] with i the group
    # slot (input group 2j+i), c channel within group, o out-ch in c8
    w3f = np.asarray(w3, np.float32).reshape(8, 128, 4, 128, 25) * WSC
    w3d = np.ascontiguousarray(
        w3f.reshape(8, 128, 2, 2, 128, 25)                       # c8,o,j,i,c,t
        .transpose(0, 2, 4, 5, 3, 1)                             # c8,j,c,t,i,o
        .reshape(8, 2, 128, 25 * 2 * 128)).astype(E4)
    wsa = np.ascontiguousarray(np.asarray(conv_w, np.float32).reshape(K, C).T).astype(BF)
    bsab = np.ascontiguousarray(
        np.broadcast_to(np.asarray(conv_b, np.float32), (128, K)))
    b1r = np.ascontiguousarray(np.asarray(b1, np.float32).reshape(2, 128).T) * WSC
    b2r = np.ascontiguousarray(np.asarray(b2, np.float32).reshape(4, 128).T) * WSC
    b3r = np.ascontiguousarray(np.asarray(b3, np.float32).reshape(8, 128).T) * WSC
    cent = np.ascontiguousarray(np.asarray(cluster_centers, np.float32))
    mlpwt = np.zeros((76, 128, 256), np.float32)
    mlpwt[:72] = np.asarray(mlp_w, np.float32).T.reshape(72, 128, 256)
    mlpwt[72, 0, :] = np.asarray(mlp_b, np.float32)
    mlpwt = np.ascontiguousarray(
        mlpwt.reshape(19, 4, 128, 256).transpose(0, 2, 1, 3).reshape(19, 128, 4 * 256)).astype(BF)

    shared = dict(w1d=w1d, w1s=w1s, w2d=w2d, w3d=w3d, wsa=wsa, bsab=bsab,
                  b1r=b1r, b2r=b2r, b3r=b3r, cent=cent, mlpwt=mlpwt)
    in_maps = []
    for core in range(NCORES):
        s0 = core * S
        m = dict(shared)
        m["xsa"] = np.ascontiguousarray(xsa[s0:s0 + S])
        m["xp8"] = np.ascontiguousarray(xp8[s0:s0 + S])
        m["xt"] = np.ascontiguousarray(xt[s0:s0 + S])
        in_maps.append(m)
    return in_maps


def kernel(**inputs):
    nc = _get_program()
    in_maps = prep_inputs(**inputs)
    res = run_bass_kernel_spmd(nc, in_maps, core_ids=list(range(NCORES)))
    return np.concatenate([r["out"] for r in res.results], axis=0)


if __name__ == "__main__":
    import reference
    ins = {k: np.asarray(v) for k, v in reference.setup_inputs().items()}
    out = kernel(**ins)
    print(out.shape, out.dtype)


# revision 27
# speedup vs baseline: 1.0072x; 1.0072x over previous
"""NetVLAD Trainium2 kernel: data-parallel over batch (2 samples/core x 8 cores).

v3: conv1 + conv2 run in fp8 e4m3 with DoubleRow perf mode (2 contraction
rows per PE cycle -> 2x matmul throughput; measured 169 ns per N=400
DR matmul vs 169 ns bf16, double the work).  conv1 pairs taps along ky
(stride-208 pairs) plus two stride-1 kx pairs on the ky=4 row; conv2
pairs the two 128-channel input groups (h1 stored group-interleaved
fp8).  Weights are pre-scaled x16 to dodge e4m3 subnormals; the 1/16
fold happens in the existing bias tensor_scalar.  conv3, soft-assign,
VLAD and the MLP stay bf16/fp32 for accuracy (measured scale-rel err
~1.1e-2 vs 2e-2 budget).
"""

import sys

sys.path.insert(0, "/opt/trn_rl_repo")

from dataclasses import replace as _ap_replace

import numpy as np
import ml_dtypes

BF = ml_dtypes.bfloat16
E4 = ml_dtypes.float8_e4m3

import concourse.bass as bass
import concourse.tile as tile
from concourse import bacc, mybir
from concourse.bass_utils import run_bass_kernel_spmd

F32 = mybir.dt.float32
BF16 = mybir.dt.bfloat16
FP8 = mybir.dt.float8e4
PM = mybir.MatmulPerfMode
ALU = mybir.AluOpType
ACT = mybir.ActivationFunctionType
AXIS = mybir.AxisListType

S = 2          # samples per core
NCORES = 8
C = 128        # input channels
K = 64         # clusters
H, W = 40, 200
P = H * W      # 8000 pixels
HP, WP = H + 4, W + 4          # padded 44 x 204 (bf16 soft-assign copy)
W8 = 208                       # fp8 conv1 copy padded to 208-wide rows
CHUNK = 100                     # pixels per vlad chunk (half of a row)
NCHUNK = P // CHUNK             # 80
NB = NCHUNK // 8                # 10 batches of 8 chunks
EPS = 1e-12
LEAK = 0.2
WSC = 16.0                     # fp8 weight pre-scale
WSCI = 1.0 / WSC

# conv1: 128 -> 256, conv2: 256 -> 512, conv3: 512 -> 1024
# h1 pooled 20x100 (padded 24x104), h2 pooled 5x25 (padded 9x32: 32-wide
# rows keep the fp8 group-pair stride 16B-aligned and let the conv3
# moving operand over-read to 26 columns)
H1, W1 = 20, 100
H1P, W1P = 24, 104
H2, W2 = 5, 25
H2P, W2P = 9, 32
FEAT_CHUNKS = 73  # 64 vlad + 8 xfeat + 1 bias row
N_EARLY = 8       # conv1 (s=0, c2=0) groups pre-SA-hook filled into phase A
# (sample, b) -> t20 for tail-position fillers (emitted before the vlad block)
# phase-A fillers: DoubleRow groups early in sample 1's SA pipeline
# hard-fault the device (bisected empirically: DR fillers pass only with
# <=1 SA batch still ahead).  So early fillers run in normal fp8 mode
# (same weights read unpaired, 25 MMs) and only the late positions use DR.
import os as _os
_XV = _os.environ.get("KXV", "x1")
if _XV == "x1":      # safe tail fills only, skip exactly those groups in B
    TAIL_FILL = {(0, 10): (4, "dr"), (0, 11): (5, "dr"),
                 (1, 9): (6, "dr"), (1, 11): (7, "dr")}
    SKIP_SET = {4, 5, 6, 7}
elif _XV == "x3":    # two groups per safe tail position
    TAIL_FILL = {(0, 10): [(4, "dr"), (0, "dr")], (0, 11): [(5, "dr"), (1, "dr")],
                 (1, 9): [(6, "dr"), (2, "dr")], (1, 11): [(7, "dr"), (3, "dr")]}
    SKIP_SET = {0, 1, 2, 3, 4, 5, 6, 7}
elif _XV == "x2":    # tail fills, no skip (duplicate emission) — known good
    TAIL_FILL = {(0, 10): (4, "dr"), (0, 11): (5, "dr"),
                 (1, 9): (6, "dr"), (1, 11): (7, "dr")}
    SKIP_SET = set()
else:                # no fills at all
    TAIL_FILL = {}
    SKIP_SET = set()

# conv1 tap pairing: 10 ky-pairs (stride W8) + 2 kx-pairs on ky=4 (stride 1)
# + 1 single (ky=4, kx=4).  pair p<10: kx=p//2, ky0=2*(p%2); p=10: (4,0)+(4,1);
# p=11: (4,2)+(4,3).


def _strided(apv, dims):
    """AP with explicit [stride, n] free dims after the partition dim."""
    return _ap_replace(
        apv, ap=type(apv.ap)([list(apv.ap[0])] + [list(d) for d in dims]))


def build_program():
    nc = bacc.Bacc("TRN2", target_bir_lowering=False)

    # ---- DRAM I/O ----
    d_xsa = nc.dram_tensor("xsa", [S, C, P], BF16, kind="ExternalInput")
    d_xp8 = nc.dram_tensor("xp8", [S, C, HP * W8], FP8, kind="ExternalInput")
    d_xt = nc.dram_tensor("xt", [S, CHUNK, NCHUNK * 130], BF16, kind="ExternalInput")
    d_w1d = nc.dram_tensor("w1d", [C, 2 * 12 * 2 * 128], FP8, kind="ExternalInput")
    d_w1s = nc.dram_tensor("w1s", [C, 2 * 128], FP8, kind="ExternalInput")
    d_w2d = nc.dram_tensor("w2d", [4, C, 25 * 2 * 128], FP8, kind="ExternalInput")
    d_w3d = nc.dram_tensor("w3d", [8, 2, C, 25 * 2 * 128], FP8, kind="ExternalInput")
    d_wsa = nc.dram_tensor("wsa", [C, K], BF16, kind="ExternalInput")
    d_bsab = nc.dram_tensor("bsab", [128, K], F32, kind="ExternalInput")
    d_b1 = nc.dram_tensor("b1r", [128, 2], F32, kind="ExternalInput")
    d_b2 = nc.dram_tensor("b2r", [128, 4], F32, kind="ExternalInput")
    d_b3 = nc.dram_tensor("b3r", [128, 8], F32, kind="ExternalInput")
    d_cent = nc.dram_tensor("cent", [K, C], F32, kind="ExternalInput")
    d_mlpw = nc.dram_tensor("mlpwt", [19, 128, 4 * 256], BF16, kind="ExternalInput")
    d_out = nc.dram_tensor("out", [S, 256], F32, kind="ExternalOutput")
    d_ident = nc.inline_tensor(np.eye(128, dtype=np.float32).astype(BF),
                               name="ident")
    _brow = np.zeros((128, 2), np.float32)
    _brow[0, :] = 1.0
    d_brow = nc.inline_tensor(_brow.astype(BF), name="brow")

    with tile.TileContext(nc) as tc:
        _body(nc, tc, d_xsa, d_xp8, d_xt, d_w1d, d_w1s, d_w2d, d_w3d,
              d_wsa, d_bsab, d_b1, d_b2, d_b3, d_cent, d_mlpw, d_out,
              d_ident, d_brow)

    nc.compile()
    return nc


def _body(nc, tc, d_xsa, d_xp8, d_xt, d_w1d, d_w1s, d_w2d, d_w3d,
          d_wsa, d_bsab, d_b1, d_b2, d_b3, d_cent, d_mlpw, d_out,
          d_ident, d_brow):
    from contextlib import ExitStack

    # ---- pool creation in stack (LIFO) order; DMAs issued afterwards in
    # priority order (x streams first on sync, consts on scalar) ----
    es_const = ExitStack()
    const = es_const.enter_context(tc.tile_pool(name="const", bufs=1))
    wsa_sb = const.tile([C, K], BF16, tag="wsa")
    bsab_sb = const.tile([128, K], F32, tag="bsab")
    cent_sb = const.tile([K, C], F32, tag="cent")
    b1_sb = const.tile([128, 2], F32, tag="b1")
    b2_sb = const.tile([128, 4], F32, tag="b2")
    b3_sb = const.tile([128, 8], F32, tag="b3")
    ident_sb = const.tile([128, 128], BF16, tag="ident")
    ones64_sb = const.tile([K, K], BF16, tag="ones64")
    ones128_sb = const.tile([128, 128], BF16, tag="ones128")
    biasrow_sb = const.tile([128, 2], BF16, tag="biasrow")
    # feature tiles for the MLP (filled by phases A and D)
    fv_sb = const.tile([128, K * S], BF16, tag="fv")      # [c, (k, s)]
    xf_sb = const.tile([128, S * 8], F32, tag="xf")      # [c, (s, c8)]

    es_ab = ExitStack()
    xpad_pool = es_ab.enter_context(tc.tile_pool(name="xpad", bufs=S))
    xp8_pool = es_ab.enter_context(tc.tile_pool(name="xp8", bufs=S))
    w1_pool = es_ab.enter_context(tc.tile_pool(name="w1p", bufs=2))
    # pools that survive into phase B (vlad posts + interleaved conv1
    # groups run alongside phase A / under conv1)
    es_apost = ExitStack()
    vpost_pool = es_apost.enter_context(tc.tile_pool(name="vpost", bufs=2))
    vlad_ps = es_apost.enter_context(
        tc.tile_pool(name="vlad_ps", bufs=2, space="PSUM"))
    tp_ps = es_apost.enter_context(tc.tile_pool(name="tp_ps", bufs=1, space="PSUM"))
    # conv1 pools created before phase A so conv1 groups can interleave
    # into phase-A stall windows (PSUM: vlad 2 + tp 2 + c1 2 + sa 2 = 8)
    c1_ps = es_apost.enter_context(tc.tile_pool(name="c1ps", bufs=2, space="PSUM"))
    pb_pool = es_apost.enter_context(tc.tile_pool(name="poolb", bufs=4))
    xt_pool_l = ExitStack()
    xt_pool = xt_pool_l.enter_context(tc.tile_pool(name="xtl", bufs=S))

    es_bd = ExitStack()  # mlp weights live from B to E (bottom of right stack)
    mw_pool = es_bd.enter_context(
        tc.tile_pool(name="mws", bufs=19, side="right"))
    es_bc = ExitStack()  # right-side pools live through B and C
    h1_pool = es_bc.enter_context(
        tc.tile_pool(name="h1", bufs=S, side="right"))
    w2_pool = es_bc.enter_context(
        tc.tile_pool(name="w2s", bufs=4, side="right"))

    # h1: per sample, both 128-channel groups interleaved for conv2
    # DoubleRow pairs: [c, (c2, pix)] fp8, pair stride H1P*W1P = 2496 B.
    # The memsets sit BEFORE the xt chunk DMAs on the gpsimd queue on
    # purpose: delaying xt ~9us serializes it behind the xpad stream,
    # which otherwise contends for HBM exactly while soft-assign is
    # racing its xpad chunk arrivals (measured +4us of PE stall).
    h1_sb = [None] * S
    for s in range(S):
        t = h1_pool.tile([128, 2 * H1P * W1P], FP8, tag="h1", name=f"h1_{s}")
        nc.gpsimd.memset(t[:], 0.0)
        h1_sb[s] = t

    # ---- input streams in consumption order: first x-pad row-chunks of
    # sample 0 + the soft-assign weights, then the rest ----
    xsa_sb = [xpad_pool.tile([C, P], BF16, tag="xsa", name=f"xp{s}")
              for s in range(S)]
    xp8_sb = [xp8_pool.tile([C, HP * W8], FP8, tag="xp8", name=f"x8{s}")
              for s in range(S)]
    xt_sb = [xt_pool.tile([CHUNK, NCHUNK * 130], BF16, tag="xt", name=f"xt{s}")
             for s in range(S)]

    # raw (unpadded) rows: soft-assign never reads the conv padding, so
    # the SA copy ships unpadded (0.3 MB/sample less on the critical
    # startup stream; SA batch b needs rows <= 4b+3)
    row_chunks = [(0, 4), (4, 15), (15, 26), (26, 36), (36, 40)]

    def xpad_chunk(s, r):
        a, b = row_chunks[r][0] * W, row_chunks[r][1] * W
        nc.sync.dma_start(xsa_sb[s][:, a:b], d_xsa[s][:, a:b])

    def xt_chunk(s, g):
        a, b = g * 20 * 130, (g + 1) * 20 * 130
        nc.gpsimd.dma_start(xt_sb[s][:, a:b], d_xt[s][:, a:b])

    # interleaved in consumption order: SA batch b needs xpad rows <= 4b+5,
    # vlad batch b needs xt group b//2.5 -- small first chunk so the first
    # soft-assign matmul issues as early as possible
    w1d_sb = w1_pool.tile([C, 2 * 12 * 2 * 128], FP8, tag="w1d")
    w1s_sb = w1_pool.tile([C, 2 * 128], FP8, tag="w1s")

    nc.sync.dma_start(wsa_sb[:], d_wsa[:])
    nc.scalar.dma_start(bsab_sb[:], d_bsab[:])
    for s in range(S):
        xpad_chunk(s, 0)
        xpad_chunk(s, 1)
        xt_chunk(s, 0)
        xpad_chunk(s, 2)
        xt_chunk(s, 1)
        xpad_chunk(s, 3)
        xt_chunk(s, 2)
        xpad_chunk(s, 4)
        xt_chunk(s, 3)
        if s == 0:
            # w1 + conv1 bias + fp8 x land before sample 1's phase A so
            # conv1 groups can interleave into its stall windows; scalar
            # queue so the xpad/xt streams keep their own bandwidth
            # (NOTE: do NOT move these onto sync behind the xpad chunks --
            # the delayed arrival re-triggers the phase-A fill device
            # fault, NRT_EXEC_UNIT_UNRECOVERABLE)
            nc.scalar.dma_start(xp8_sb[0][:], d_xp8[0][:])
            nc.scalar.dma_start(w1d_sb[:], d_w1d[:])
            nc.scalar.dma_start(w1s_sb[:], d_w1s[:])
            nc.scalar.dma_start(b1_sb[:], d_b1[:])
    nc.scalar.dma_start(cent_sb[:], d_cent[:])
    nc.scalar.dma_start(b2_sb[:], d_b2[:])
    nc.scalar.dma_start(b3_sb[:], d_b3[:])
    nc.scalar.dma_start(ident_sb[:], d_ident[:])
    nc.gpsimd.memset(ones64_sb[:], 1.0)
    nc.gpsimd.memset(ones128_sb[:], 1.0)
    nc.scalar.dma_start(biasrow_sb[:], d_brow[:])

    w1dv = w1d_sb[:].rearrange("p (g t two o) -> p g t two o", g=2, t=12, two=2)
    w1sv = w1s_sb[:].rearrange("p (g o) -> p g o", g=2)

    # one conv1 output group (2 output rows): 12 DoubleRow fp8 pair-matmuls
    # + 1 single-tap fp8 matmul, 2x2 maxpool, bias (x16 weights folded back
    # with the 1/16), leaky, store into group-interleaved fp8 h1.
    def conv1_group(s, c2, t20, mode="dr"):
        xv8 = xp8_sb[s][:].rearrange("p (h w) -> p h w", h=HP)
        r0 = 2 * t20
        ps = c1_ps.tile([128, 400], F32, tag="c1")
        if mode == "dr":
            for p in range(12):
                if p < 10:
                    kx, ky0 = p // 2, 2 * (p % 2)
                    base = xv8[:, r0 + ky0:r0 + ky0 + 1, kx:kx + 1]
                    rhs = _strided(base, [[W8, 2], [W8, 2], [1, 200]])
                else:
                    kx0 = (p - 10) * 2
                    base = xv8[:, r0 + 4:r0 + 5, kx0:kx0 + 1]
                    rhs = _strided(base, [[1, 2], [W8, 2], [1, 200]])
                nc.tensor.matmul(ps[:], w1dv[:, c2, p], rhs,
                                 start=(p == 0), stop=False,
                                 perf_mode=PM.DoubleRow)
        else:
            for p in range(12):
                for i in range(2):
                    if p < 10:
                        ky, kx = 2 * (p % 2) + i, p // 2
                    else:
                        ky, kx = 4, (p - 10) * 2 + i
                    nc.tensor.matmul(
                        ps[:], w1dv[:, c2, p, i],
                        xv8[:, r0 + ky:r0 + ky + 2, kx:kx + 200],
                        start=(p == 0 and i == 0), stop=False)
        nc.tensor.matmul(ps[:], w1sv[:, c2], xv8[:, r0 + 4:r0 + 6, 4:204],
                         start=False, stop=True)
        # maxpool 2x2 -> [128, 100], then bias + leaky
        m1 = pb_pool.tile([128, 200], F32, tag="m1")
        nc.vector.tensor_reduce(
            m1[:], ps[:].rearrange("p (a two) -> p a two", two=2),
            axis=AXIS.X, op=ALU.max)
        m2 = pb_pool.tile([128, 100], F32, tag="m2")
        mv = m1[:].rearrange("p (r x) -> p r x", r=2)
        nc.vector.tensor_tensor(
            m2[:].rearrange("p (r x) -> p r x", r=1),
            mv[:, 0:1, :], mv[:, 1:2, :], op=ALU.max)
        nc.vector.tensor_scalar(
            m2[:], m2[:], b1_sb[:, c2:c2 + 1], WSCI, op0=ALU.add, op1=ALU.mult)
        dst = h1_sb[s][:, c2 * (H1P * W1P) + (t20 + 2) * W1P + 2:
                       c2 * (H1P * W1P) + (t20 + 2) * W1P + 2 + 100]
        nc.vector.scalar_tensor_tensor(
            dst, m2[:], LEAK, m2[:], op0=ALU.mult, op1=ALU.max)

    # ---------------- Phase A: soft-assign + VLAD ----------------
    es_a = ExitStack()
    ea_pool = es_a.enter_context(tc.tile_pool(name="ea", bufs=4))
    u_pool = es_a.enter_context(tc.tile_pool(name="u", bufs=4))
    sa_ps = es_a.enter_context(tc.tile_pool(name="sa_ps", bufs=2, space="PSUM"))

    vps_l = [None] * S
    for s in range(S):
        xpv = xsa_sb[s]
        xtv = xt_sb[s]
        vps = vlad_ps.tile([K, 132], F32, tag="vlad")
        vps_l[s] = vps
        # 2-deep software pipeline: soft-assign batch b, vlad batch b-2, so
        # the ~2us softmax chain latency hides under two batches of PE work
        a_hist = {}
        for b in range(NB + 2):
            # tail filler BEFORE the vlad block: fills xt/softmax stalls in
            # the thin tail iterations (only 8 vlad matmuls of PE work each)
            if (s, b) in TAIL_FILL:
                fills = TAIL_FILL[(s, b)]
                if not isinstance(fills, list):
                    fills = [fills]
                for ft, fm in fills:
                    conv1_group(0, 0, ft, mode=fm)
            if b < NB:
                saps = sa_ps.tile([128, 512], F32, tag="sa")
                for i8 in range(8):
                    ci = b * 8 + i8
                    y, half = divmod(ci, 2)
                    off = y * W + half * CHUNK
                    nc.tensor.matmul(
                        saps[0:CHUNK, i8 * K:(i8 + 1) * K],
                        xpv[:, off:off + CHUNK],
                        wsa_sb[:],
                        start=True, stop=True,
                    )
                e_t = ea_pool.tile([128, 512], F32, tag="e")
                nc.vector.tensor_tensor(
                    e_t[0:CHUNK].rearrange("p (a b) -> p a b", a=8),
                    saps[0:CHUNK].rearrange("p (a b) -> p a b", a=8),
                    bsab_sb[0:CHUNK].rearrange("p (o k) -> p o k", o=1).broadcast_to((CHUNK, 8, K)),
                    op=ALU.add,
                )
                nc.scalar.activation(e_t[0:CHUNK], e_t[0:CHUNK], ACT.Exp)
                ssum = u_pool.tile([128, 8], F32, tag="ssum")
                nc.vector.tensor_reduce(
                    ssum[0:CHUNK], e_t[0:CHUNK].rearrange("p (a b) -> p a b", a=8),
                    axis=AXIS.X, op=ALU.add,
                )
                u_t = u_pool.tile([128, 8], F32, tag="u")
                nc.vector.reciprocal(u_t[0:CHUNK], ssum[0:CHUNK])
                a_t = ea_pool.tile([128, 512], BF16, tag="a")
                nc.vector.tensor_tensor(
                    a_t[0:CHUNK].rearrange("p (a b) -> p a b", a=8),
                    e_t[0:CHUNK].rearrange("p (a b) -> p a b", a=8),
                    u_t[0:CHUNK].rearrange("p (a o) -> p a o", o=1).broadcast_to((CHUNK, 8, K)),
                    op=ALU.mult,
                )
                a_hist[b] = a_t
            if b >= 2:
                bb = b - 2
                at = a_hist.pop(bb)
                for i8 in range(8):
                    ci = bb * 8 + i8
                    nc.tensor.matmul(
                        vps[0:K, 0:130],
                        at[0:CHUNK, i8 * K:(i8 + 1) * K],
                        xtv[:, ci * 130:(ci + 1) * 130],
                        start=(ci == 0), stop=(ci == NCHUNK - 1),
                    )
    # vlad post for both samples, deferred into phase B: the PE ops (gps,
    # transpose) queue behind a few conv1 groups so their DVE input chains
    # complete long before the PE reaches them
    def do_vlad_post(s):
        vps = vps_l[s]
        # vlad post: v' = centers*A - vlad1  (negated vlad)
        vp = vpost_pool.tile([K, C], F32, tag="vp")
        acol = vpost_pool.tile([K, 4], F32, tag="acol")
        nc.vector.tensor_copy(acol[:, 0:1], vps[0:K, 128:129])
        nc.vector.scalar_tensor_tensor(
            vp[:], cent_sb[:], acol[:, 0:1], vps[0:K, 0:C],
            op0=ALU.mult, op1=ALU.subtract,
        )
        sq = vpost_pool.tile([K, C], F32, tag="vsq")
        ssk = vpost_pool.tile([K, 4], F32, tag="ssk")
        nc.scalar.activation(sq[:], vp[:], ACT.Square, accum_out=ssk[:, 0:1])
        nc.scalar.sqrt(ssk[:, 1:2], ssk[:, 0:1])
        nc.vector.tensor_scalar_max(ssk[:, 1:2], ssk[:, 1:2], EPS)
        nc.vector.reciprocal(ssk[:, 2:3], ssk[:, 1:2])
        # per-row ss of the normalized rows = ssk * ik^2
        nc.vector.tensor_scalar(
            ssk[:, 3:4], ssk[:, 0:1], ssk[:, 2:3], None, op0=ALU.mult)
        nc.vector.tensor_scalar(
            ssk[:, 3:4], ssk[:, 3:4], ssk[:, 2:3], None, op0=ALU.mult)
        sskb = vpost_pool.tile([K, 2], BF16, tag="sskb")
        nc.vector.tensor_copy(sskb[:, 0:1], ssk[:, 3:4])
        gps = tp_ps.tile([K, 4], F32, tag="gps")
        nc.tensor.matmul(gps[:, 0:1], ones64_sb[:], sskb[:, 0:1],
                         start=True, stop=True)
        gsb = vpost_pool.tile([K, 4], F32, tag="gsb")
        nc.scalar.sqrt(gsb[:, 0:1], gps[:, 0:1])
        nc.vector.tensor_scalar_max(gsb[:, 0:1], gsb[:, 0:1], EPS)
        nc.vector.reciprocal(gsb[:, 1:2], gsb[:, 0:1])
        # combined scale = -ik * ginv
        nc.vector.tensor_scalar(
            gsb[:, 2:3], ssk[:, 2:3], gsb[:, 1:2], -1.0,
            op0=ALU.mult, op1=ALU.mult)
        vf = vpost_pool.tile([K, C], BF16, tag="vf")
        nc.vector.tensor_scalar(vf[:], vp[:], gsb[:, 2:3], None, op0=ALU.mult)
        # transpose [64, 128] -> [128, 64] and store into fv[:, (k, s)]
        vtps = tp_ps.tile([128, K], BF16, tag="vt")
        nc.tensor.transpose(vtps[:], vf[:], ident_sb[0:K, 0:K])
        nc.vector.tensor_copy(
            fv_sb[:].rearrange("p (k s) -> p k s", s=S)[:, :, s],
            vtps[:],
        )
    # xp8[1] is first consumed mid-phase-B: its doorbell sits after the
    # phase-A loop so the scalar engine (busy with softmax Exp) only
    # fires it ~55us in, keeping startup HBM bandwidth for the SA stream
    nc.scalar.dma_start(xp8_sb[1][:], d_xp8[1][:])
    es_a.close()
    xt_pool_l.close()

    # ---------------- Phase B: conv1 + pool 2x2 + leaky ----------------
    w2_sb = {}

    def load_w2(c4):
        t = w2_pool.tile([128, 25 * 2 * 128], FP8, tag="w2s")
        nc.sync.dma_start(t[:], d_w2d[c4])
        w2_sb[c4] = t

    load_w2(0)

    mw_sb = {}

    def load_mw(g):
        t = mw_pool.tile([128, 4 * 256], BF16, tag="mws")
        nc.sync.dma_start(t[:], d_mlpw[g])
        mw_sb[g] = t

    for s in range(S):
        if s == 1:
            for g in range(19):
                load_mw(g)
        for c2 in range(2):
            for t20 in range(20):
                if s == 0 and c2 == 0 and t20 in SKIP_SET:
                    continue  # already emitted interleaved into phase A
                if s == 0 and c2 == 0 and t20 == N_EARLY + 3:
                    do_vlad_post(0)
                    do_vlad_post(1)
                conv1_group(s, c2, t20)
    es_apost.close()
    es_ab.close()

    # ---------------- Phase C: conv2 + pool 4x4 + leaky ----------------
    # one PSUM pool shared by conv2, conv3 and the MLP so bank rotation
    # pipelines across the phase boundaries (no bank-drain wait at C->D->E)
    es_cde = ExitStack()
    cd_ps = es_cde.enter_context(tc.tile_pool(name="cdps", bufs=6, space="PSUM"))
    mlp_ps = es_cde.enter_context(tc.tile_pool(name="mlpps", bufs=1, space="PSUM"))
    nm_ps = es_cde.enter_context(tc.tile_pool(name="nmps", bufs=1, space="PSUM"))
    es_cd = ExitStack()  # left-side pools live through C and D
    h2_pool = es_cd.enter_context(tc.tile_pool(name="h2", bufs=4))
    w3_pool = es_cd.enter_context(tc.tile_pool(name="w3s", bufs=16))
    es_c = ExitStack()
    pc_pool = es_c.enter_context(tc.tile_pool(name="poolc", bufs=4))

    # h2: tile j holds conv2 output groups (2j, 2j+1) as DoubleRow pairs
    # for conv3; pair stride S*H2P*W2P = 576 B
    h2_sb = [None] * 2
    for j in range(2):
        t = h2_pool.tile([128, 2 * S * H2P * W2P], FP8, tag="h2")
        nc.gpsimd.memset(t[:], 0.0)
        h2_sb[j] = t

    w3_sb = {}

    def load_w3(c8):
        for j in range(2):
            t = w3_pool.tile([128, 25 * 2 * 128], FP8, tag="w3s")
            nc.sync.dma_start(t[:], d_w3d[c8, j])
            w3_sb[(c8, j)] = t

    for c4 in range(4):
        if c4 + 1 < 4:
            load_w2(c4 + 1)
        if c4 == 2:
            load_w3(0)
        elif c4 == 3:
            load_w3(1)
            load_w3(2)
        w2v = w2_sb[c4][:].rearrange("p (t two o) -> p t two o", t=25, two=2)
        for s in range(S):
            for rg in range(5):
                ps = cd_ps.tile([128, 400], F32, tag="c23")
                for tap in range(25):
                    ky, kx = divmod(tap, 5)
                    base = h1_sb[s][:, (4 * rg + ky) * W1P + kx:
                                    (4 * rg + ky) * W1P + kx + 1]
                    rhs = _strided(
                        base, [[H1P * W1P, 2], [W1P, 4], [1, 100]])
                    nc.tensor.matmul(
                        ps[:], w2v[:, tap], rhs,
                        start=(tap == 0), stop=(tap == 24),
                        perf_mode=PM.DoubleRow,
                    )
                # maxpool 4x4 over [4 rows, 100] -> [128, 25]
                ma = pc_pool.tile([128, 200], F32, tag="ma")
                nc.vector.tensor_reduce(
                    ma[:], ps[:].rearrange("p (a two) -> p a two", two=2),
                    axis=AXIS.X, op=ALU.max)
                mb = pc_pool.tile([128, 100], F32, tag="mb")
                mav = ma[:].rearrange("p (a two) -> p a two", two=2)
                nc.vector.tensor_tensor(
                    mb[:].rearrange("p (a o) -> p a o", o=1),
                    mav[:, :, 0:1], mav[:, :, 1:2], op=ALU.max)
                mc = pc_pool.tile([128, 50], F32, tag="mc")
                mbv = mb[:].rearrange("p (r x) -> p r x", r=4)
                mcv = mc[:].rearrange("p (r x) -> p r x", r=2)
                nc.vector.tensor_tensor(
                    mcv[:, 0:1, :], mbv[:, 0:1, :], mbv[:, 1:2, :], op=ALU.max)
                nc.vector.tensor_tensor(
                    mcv[:, 1:2, :], mbv[:, 2:3, :], mbv[:, 3:4, :], op=ALU.max)
                md = pc_pool.tile([128, 25], F32, tag="md")
                nc.vector.tensor_tensor(
                    md[:].rearrange("p (r x) -> p r x", r=1),
                    mcv[:, 0:1, :], mcv[:, 1:2, :], op=ALU.max)
                nc.vector.tensor_scalar(
                    md[:], md[:], b2_sb[:, c4:c4 + 1], WSCI,
                    op0=ALU.add, op1=ALU.mult)
                off = (c4 % 2) * (S * H2P * W2P) \
                    + (rg + 2) * (S * W2P) + s * W2P + 2
                nc.vector.scalar_tensor_tensor(
                    h2_sb[c4 // 2][:, off:off + 25], md[:], LEAK, md[:],
                    op0=ALU.mult, op1=ALU.max)
    es_c.close()
    es_bc.close()

    # ---------------- Phase D: conv3 + pool 5x25 ----------------
    es_d = ExitStack()
    pd_pool = es_d.enter_context(tc.tile_pool(name="poold", bufs=4))

    for c8 in range(8):
        if c8 + 3 < 8:
            load_w3(c8 + 3)
        # h2 rows are sample-interleaved (row stride S*W2P, sample W2P),
        # so (row, sample) merges into ONE 10-step stride-32 AP dim and a
        # single N=260 matmul serves both samples per pair-tap: the 135ns
        # DR weight load is the only bound (two N=130 MMs measured
        # 82.5 ns/MM vs the 67.5 floor -- ~12us of second-MM overhead)
        ps_t = cd_ps.tile([128, 400], F32, tag="c23")
        ps = ps_t[:, 0:260]
        for j in range(2):
            wv = w3_sb[(c8, j)][:].rearrange("p (t two o) -> p t two o",
                                             t=25, two=2)
            for tap in range(25):
                ky, kx = divmod(tap, 5)
                base = h2_sb[j][:, ky * (S * W2P) + kx:
                                ky * (S * W2P) + kx + 1]
                rhs = _strided(
                    base, [[S * H2P * W2P, 2], [W2P, S * 5], [1, 26]])
                nc.tensor.matmul(
                    ps, wv[:, tap], rhs,
                    start=(j == 0 and tap == 0),
                    stop=(j == 1 and tap == 24),
                    perf_mode=PM.DoubleRow,
                )
        pv = ps.rearrange("p (h s w) -> p h s w", h=5, s=S)
        for s in range(S):
            mx = pd_pool.tile([128, 4], F32, tag="mx")
            nc.vector.tensor_reduce(
                mx[:, 0:1], pv[:, :, s, 0:25], axis=AXIS.XY, op=ALU.max)
            nc.vector.tensor_scalar(
                xf_sb[:].rearrange("p (s c) -> p s c", s=S)[:, s, c8:c8 + 1],
                mx[:, 0:1], b3_sb[:, c8:c8 + 1], WSCI,
                op0=ALU.add, op1=ALU.mult)
    es_d.close()
    es_cd.close()

    # ---------------- Phase E: x_feat norm + MLP + final norm ----------------
    es_e = ExitStack()
    pe_pool = es_e.enter_context(tc.tile_pool(name="poole", bufs=1))

    # vlad-part MLP chunks first: they only need fv, so the PE starts on
    # them right after conv3; the x_feat norm chain (DVE/ACT + one small
    # matmul) is emitted mid-loop so it overlaps the remaining chunks
    ops = mlp_ps.tile([S, 256], F32, tag="mlpo")
    fvv = fv_sb[:].rearrange("p (k s) -> p k s", s=S)
    for j in range(K // 2):
        g, i4 = divmod(j, 4)
        nc.tensor.matmul(
            ops[:],
            fvv[:, j, :],
            mw_sb[g][:, i4 * 256:(i4 + 1) * 256],
            start=(j == 0), stop=False,
        )

    # x_feat l2 norm across the 1024 conv3 features of each sample
    sq = pe_pool.tile([128, S * 8], BF16, tag="sq")
    nc.vector.tensor_tensor(sq[:], xf_sb[:], xf_sb[:], op=ALU.mult)
    sps = nm_ps.tile([128, S * 8], F32, tag="sps")
    nc.tensor.matmul(sps[:], ones128_sb[:], sq[:], start=True, stop=True)

    for j in range(K // 2, K):
        g, i4 = divmod(j, 4)
        nc.tensor.matmul(
            ops[:],
            fvv[:, j, :],
            mw_sb[g][:, i4 * 256:(i4 + 1) * 256],
            start=False, stop=False,
        )

    ssn = pe_pool.tile([128, S * 4], F32, tag="ssn")
    nc.vector.tensor_reduce(
        ssn[:, 0:S], sps[:].rearrange("p (s c) -> p s c", s=S),
        axis=AXIS.X, op=ALU.add)
    nc.scalar.sqrt(ssn[:, S:2 * S], ssn[:, 0:S])
    nc.vector.tensor_scalar_max(ssn[:, S:2 * S], ssn[:, S:2 * S], EPS)
    nc.vector.reciprocal(ssn[:, 2 * S:3 * S], ssn[:, S:2 * S])
    xff = pe_pool.tile([128, S * 8], BF16, tag="xff")
    nc.vector.tensor_tensor(
        xff[:].rearrange("p (s c) -> p s c", s=S),
        xf_sb[:].rearrange("p (s c) -> p s c", s=S),
        ssn[:, 2 * S:3 * S].rearrange("p (s o) -> p s o", s=S).broadcast_to((128, S, 8)),
        op=ALU.mult)

    xfv = xff[:].rearrange("p (s c) -> p s c", s=S)
    for j in range(K, FEAT_CHUNKS):
        lhs = xfv[:, :, j - K] if j < K + 8 else biasrow_sb[:]
        g, i4 = divmod(j, 4)
        nc.tensor.matmul(
            ops[:],
            lhs,
            mw_sb[g][:, i4 * 256:(i4 + 1) * 256],
            start=False, stop=(j == FEAT_CHUNKS - 1),
        )
    sqo = pe_pool.tile([S, 256], F32, tag="sqo")
    nrm = pe_pool.tile([S, 4], F32, tag="nrm")
    nc.scalar.activation(sqo[:], ops[:], ACT.Square, accum_out=nrm[:, 0:1])
    nc.scalar.sqrt(nrm[:, 1:2], nrm[:, 0:1])
    nc.vector.tensor_scalar_max(nrm[:, 1:2], nrm[:, 1:2], EPS)
    nc.vector.reciprocal(nrm[:, 2:3], nrm[:, 1:2])
    out_sb = pe_pool.tile([S, 256], F32, tag="outsb")
    nc.vector.tensor_scalar(
        out_sb[:], ops[:], nrm[:, 2:3], None, op0=ALU.mult)
    nc.sync.dma_start(d_out[:], out_sb[:])
    es_e.close()
    es_cde.close()
    es_bd.close()
    es_const.close()


_PROGRAM = None


def _get_program():
    global _PROGRAM
    if _PROGRAM is None:
        _PROGRAM = build_program()
    return _PROGRAM


def prep_inputs(x, cluster_centers, conv_w, conv_b, w1, b1, w2, b2, w3, b3,
                mlp_w, mlp_b):
    """Host-side re-layout. Returns per-core input dict list."""
    N = x.shape[0]
    x = np.asarray(x, np.float32)
    xsa = x.reshape(N, C, P).astype(BF)
    # fp8 conv1 copy, rows padded to 208 (DoubleRow pair stride alignment)
    xp8 = np.pad(x, ((0, 0), (0, 0), (2, 2), (2, 6))).reshape(N, C, HP * W8)
    xp8 = xp8.astype(E4)
    # xt: [N, 100, 80*130] pixel-transposed x with ones column
    xt = np.ascontiguousarray(x.transpose(0, 2, 3, 1))           # [N, 40, 200, 128]
    xt = xt.reshape(N, NCHUNK, CHUNK, C)                         # chunk = (y, half)
    pad_cols = np.zeros((N, NCHUNK, CHUNK, 2), np.float32)
    pad_cols[..., 0] = 1.0
    xt = np.concatenate([xt, pad_cols], axis=3)                  # [N, 80, 100, 130]
    xt = np.ascontiguousarray(
        xt.transpose(0, 2, 1, 3).reshape(N, CHUNK, NCHUNK * 130)).astype(BF)

    # conv1 DoubleRow weight pairs: w1d[c, c2, p, i, o]; taps per pair p:
    # p<10: (2*(p%2)+i, p//2); p=10: (4, i); p=11: (4, 2+i). single: (4,4).
    w1f = np.asarray(w1, np.float32).reshape(2, 128, C, 5, 5) * WSC
    w1d = np.zeros((C, 2, 12, 2, 128), np.float32)
    for p in range(12):
        for i in range(2):
            if p < 10:
                ky, kx = 2 * (p % 2) + i, p // 2
            else:
                ky, kx = 4, (p - 10) * 2 + i
            w1d[:, :, p, i, :] = w1f[:, :, :, ky, kx].transpose(2, 0, 1)
    w1s = np.ascontiguousarray(
        w1f[:, :, :, 4, 4].transpose(2, 0, 1)).astype(E4)        # [C, 2, 128]
    w1d = np.ascontiguousarray(w1d.reshape(C, -1)).astype(E4)

    # conv2 DoubleRow: w2d[c4, c_in_grp, tap, grp, o]
    w2f = np.asarray(w2, np.float32).reshape(4, 128, 2, 128, 5, 5) * WSC
    w2d = np.ascontiguousarray(
        w2f.transpose(3, 4, 5, 2, 0, 1)                          # [c,ky,kx,grp,c4,o]
        .reshape(128, 25, 2, 4, 128).transpose(3, 0, 1, 2, 4)
        .reshape(4, 128, 25 * 2 * 128)).astype(E4)
    # conv3 DoubleRow pairs: w3d[c8, j, c, tap, i, o] with i the group
    # slot (input group 2j+i), c channel within group, o out-ch in c8
    w3f = np.asarray(w3, np.float32).reshape(8, 128, 4, 128, 25) * WSC
    w3d = np.ascontiguousarray(
        w3f.reshape(8, 128, 2, 2, 128, 25)                       # c8,o,j,i,c,t
        .transpose(0, 2, 4, 5, 3, 1)                             # c8,j,c,t,i,o
        .reshape(8, 2, 128, 25 * 2 * 128)).astype(E4)
    wsa = np.ascontiguousarray(np.asarray(conv_w, np.float32).reshape(K, C).T).astype(BF)
    bsab = np.ascontiguousarray(
        np.broadcast_to(np.asarray(conv_b, np.float32), (128, K)))
    b1r = np.ascontiguousarray(np.asarray(b1, np.float32).reshape(2, 128).T) * WSC
    b2r = np.ascontiguousarray(np.asarray(b2, np.float32).reshape(4, 128).T) * WSC
    b3r = np.ascontiguousarray(np.asarray(b3, np.float32).reshape(8, 128).T) * WSC
    cent = np.ascontiguousarray(np.asarray(cluster_centers, np.float32))
    mlpwt = np.zeros((76, 128, 256), np.float32)
    mlpwt[:72] = np.asarray(mlp_w, np.float32).T.reshape(72, 128, 256)
    mlpwt[72, 0, :] = np.asarray(mlp_b, np.float32)
    mlpwt = np.ascontiguousarray(
        mlpwt.reshape(19, 4, 128, 256).transpose(0, 2, 1, 3).reshape(19, 128, 4 * 256)).astype(BF)

    shared = dict(w1d=w1d, w1s=w1s, w2d=w2d, w3d=w3d, wsa=wsa, bsab=bsab,
                  b1r=b1r, b2r=b2r, b3r=b3r, cent=cent, mlpwt=mlpwt)
    in_maps = []
    for core in range(NCORES):
        s0 = core * S
        m = dict(shared)
        m["xsa"] = np.ascontiguousarray(xsa[s0:s0 + S])
        m["xp8"] = np.ascontiguousarray(xp8[s0:s0 + S])
        m["xt"] = np.ascontiguousarray(xt[s0:s0 + S])
        in_maps.append(m)
    return in_maps


def kernel(**inputs):
    nc = _get_program()
    in_maps = prep_inputs(**inputs)
    res = run_bass_kernel_spmd(nc, in_maps, core_ids=list(range(NCORES)))
    return np.concatenate([r["out"] for r in res.results], axis=0)


if __name__ == "__main__":
    import reference
    ins = {k: np.asarray(v) for k, v in reference.setup_inputs().items()}
    out = kernel(**ins)
    print(out.shape, out.dtype)
